# revision 12
# baseline (speedup 1.0000x reference)
"""Trainium2 Bass kernel for nn_ConvchannelAttentionBlock.

reference (per batch b):
    S      = x @ x.T                      (C x C, symmetric; contraction over L)
    probs  = softmax(rowmax(S) - S)       == exp(rowmin(S) - S) / rowsum(...)
    read   = probs @ x                    (C x L)
    out    = eta * read + x

Sharding: data-parallel over B. Each of the 8 cores gets 4 batches and
runs an identical NEFF (SPMD) on its shard; outputs are concatenated.

fp8 pipeline (per batch), software-pipelined across batches by the Tile
scheduler; all matmuls run in float8e4 with MatmulPerfMode.DoubleRow
(256-deep contraction per instruction, 2x PE throughput vs bf16):
  1. DMA x (f32) in [128,1024] tiles -> SBUF; cast f32->fp8 into X8
     ([128, cm, L], row-block index on dim1) on DVE/ACT/GPSIMD.
  2. Build XT ([128, lk, C], l-block index on dim1, fp8) via PE
     transposes (128x128 blocks, 8 per PSUM stage) -> SBUF.
  3. mm1: upper-triangular blocks of S = x @ x.T via DoubleRow matmuls
     over 16 k-pairs into PSUM (f32); rounded to bf16 in SBUF; lower
     blocks mirrored via bf16 PE transposes.
  4. softmax: rowmin on DVE, E = exp(rowmin - S) on ACT (bf16) with
     fused row-sum Z; s = eta/Z; F = s*E cast to fp8 on ACT. (eta == 0
     gives F == 0 exactly, so out == x exactly.)
  5. F^T via fp8 PE transposes -> FT ([128, cm, C], d-block on dim1).
  6. mm2: R = F^T.T @ X8 via DoubleRow matmuls (2 k-pairs) into PSUM.
  7. out = R + x: half the tiles DVE-add (PSUM+SBUF), half ACT-copy +
     GPSIMD-add; staged [128,1024] f32, DMA out.
"""

import sys

if "/opt/trn_rl_repo" not in sys.path:
    sys.path.insert(0, "/opt/trn_rl_repo")

import numpy as np
import ml_dtypes

import concourse.bacc as bacc
import concourse.tile as tile
from concourse import mybir

B, C, L = 32, 512, 4096
N_CORES = 8
NB = B // N_CORES  # batches per core
P = 128            # partitions
NT = 512           # matmul moving free dim / PSUM bank (f32)
CH = 1024          # xf load/cast chunk

_F32 = mybir.dt.float32
_BF16 = mybir.dt.bfloat16
_FP8 = mybir.dt.float8e4
_DR = mybir.MatmulPerfMode.DoubleRow


def build_nc(nb=NB, c=C, l=L):
    """Build the per-core fp8 Bass kernel (nb batches of [c, l])."""
    cm = c // P            # 4 row blocks
    lk = l // P            # 32 l blocks
    ln = l // NT           # 8 mm2 column tiles
    nch = l // CH          # 4 chunks per row block
    TG = 8                 # transposes grouped per PSUM stage

    nc = bacc.Bacc("TRN2", target_bir_lowering=False, debug=False)
    x_d = nc.dram_tensor("x", [nb, c, l], _F32, kind="ExternalInput").ap()
    eta_d = nc.dram_tensor("eta128", [P, 1], _F32, kind="ExternalInput").ap()
    id8_d = nc.dram_tensor("ident8", [P, P], _FP8, kind="ExternalInput").ap()
    id16_d = nc.dram_tensor("ident16", [P, P], _BF16,
                            kind="ExternalInput").ap()
    out_d = nc.dram_tensor("out", [nb, c, l], _F32, kind="ExternalOutput").ap()

    with tile.TileContext(nc) as tc:
        with (
            tc.tile_pool(name="const", bufs=1) as const_pool,
            tc.tile_pool(name="xf", bufs=24) as xf_pool,
            tc.tile_pool(name="x8", bufs=2) as x8_pool,
            tc.tile_pool(name="xT", bufs=2) as xT_pool,
            tc.tile_pool(name="ssb", bufs=6) as ssb_pool,
            tc.tile_pool(name="ee", bufs=3) as e_pool,
            tc.tile_pool(name="ff", bufs=6) as f_pool,
            tc.tile_pool(name="ft", bufs=2) as ft_pool,
            tc.tile_pool(name="stg", bufs=4) as st_pool,
            tc.tile_pool(name="stat", bufs=20) as stat_pool,
            tc.tile_pool(name="pT", bufs=2, space="PSUM") as pT_pool,
            tc.tile_pool(name="pS", bufs=2, space="PSUM") as pS_pool,
            tc.tile_pool(name="pE", bufs=1, space="PSUM") as pE_pool,
            tc.tile_pool(name="pR", bufs=3, space="PSUM") as pR_pool,
        ):
            ident8 = const_pool.tile([P, P], _FP8, tag="id8")
            nc.sync.dma_start(ident8[:], id8_d[:, :])
            ident16 = const_pool.tile([P, P], _BF16, tag="id16")
            nc.sync.dma_start(ident16[:], id16_d[:, :])
            eta = const_pool.tile([P, 1], _F32, tag="eta")
            nc.sync.dma_start(eta[:], eta_d[:, :])

            state = {}

            def emit_load_cast(b):
                xf = [[None] * nch for _ in range(cm)]
                x8 = x8_pool.tile([P, cm, l], _FP8, tag="x8", name=f"x8_{b}")
                ci = 0
                for m in range(cm):
                    for h in range(nch):
                        t = xf_pool.tile([P, CH], _F32, tag="xf",
                                         name=f"xf_{b}_{m}_{h}")
                        nc.sync.dma_start(
                            t[:], x_d[b, m * P:(m + 1) * P,
                                      h * CH:(h + 1) * CH])
                        xf[m][h] = t
                        dst = x8[:, m, h * CH:(h + 1) * CH]
                        # gpsimd fp8 casts measured 2.5x slower than DVE/ACT
                        if ci % 2 == 0:
                            nc.vector.tensor_copy(dst, t[:])
                        else:
                            nc.scalar.copy(dst, t[:])
                        ci += 1
                state[b] = {"xf": xf, "x8": x8}

            def xf_slice(b, m, lo, width):
                h = lo // CH
                assert (lo + width - 1) // CH == h
                return state[b]["xf"][m][h][:, lo - h * CH:lo - h * CH + width]

            def emit_transpose(b):
                x8 = state[b]["x8"]
                XT = xT_pool.tile([P, lk, c], _FP8, tag="xT", name=f"XT_{b}")
                ti = 0
                for m in range(cm):
                    for j in range(lk // TG):
                        # fp8 PE transpose writes PSUM at element step 2
                        pt = pT_pool.tile([P, TG, P, 2], _FP8, tag="pT")
                        for i in range(TG):
                            kb = TG * j + i
                            nc.tensor.transpose(
                                pt[:, i, :, 0],
                                x8[:, m, kb * P:(kb + 1) * P],
                                ident8[:],
                            )
                        src = pt[:, :, :, 0]
                        dst = XT[:, TG * j:TG * (j + 1), m * P:(m + 1) * P]
                        if ti % 2 == 0:
                            nc.vector.tensor_copy(dst, src)
                        else:
                            nc.scalar.copy(dst, src)
                        ti += 1
                state[b]["XT"] = XT

            def emit_mm1_softmax(b):
                XT = state[b]["XT"]
                # S is symmetric: compute only upper-triangular blocks
                # (cols >= m*P for row-block m), round to bf16 in SBUF, and
                # fill lower blocks by PE-transposing the mirrored ones.
                S_sb = [ssb_pool.tile([P, c], _BF16, tag="ssb",
                                      name=f"Ssb_{b}_{m}")
                        for m in range(cm)]
                F = []
                for m in range(cm):
                    lo = m * P
                    ps = pS_pool.tile([P, NT], _F32, tag="pS")
                    for t in range(lk // 2):
                        nc.tensor.matmul(
                            ps[:, lo:c],
                            XT[:, 2 * t:2 * t + 2, m * P:(m + 1) * P],
                            XT[:, 2 * t:2 * t + 2, lo:c],
                            start=(t == 0),
                            stop=(t == lk // 2 - 1),
                            perf_mode=_DR,
                        )
                    nc.scalar.copy(S_sb[m][:, lo:c], ps[:, lo:c])
                    # mirror block (m, m2) for every later row-block m2
                    for m2 in range(m + 1, cm):
                        ptx = pE_pool.tile([P, P], _BF16, tag="pE")
                        nc.tensor.transpose(
                            ptx[:],
                            S_sb[m][:, m2 * P:(m2 + 1) * P],
                            ident16[:],
                        )
                        nc.scalar.copy(S_sb[m2][:, lo:lo + P], ptx[:])
                for m in range(cm):
                    mn = stat_pool.tile([P, 1], _F32, tag="stat")
                    nc.vector.tensor_reduce(
                        mn[:], S_sb[m][:], axis=mybir.AxisListType.X,
                        op=mybir.AluOpType.min)
                    e_t = e_pool.tile([P, c], _BF16, tag="ee")
                    z_t = stat_pool.tile([P, 1], _F32, tag="stat")
                    nc.scalar.activation(
                        e_t[:], S_sb[m][:], mybir.ActivationFunctionType.Exp,
                        bias=mn[:], scale=-1.0, accum_out=z_t[:])
                    r_t = stat_pool.tile([P, 1], _F32, tag="stat")
                    nc.vector.reciprocal(r_t[:], z_t[:])
                    s_t = stat_pool.tile([P, 1], _F32, tag="stat")
                    nc.vector.tensor_tensor(
                        s_t[:], eta[:], r_t[:], op=mybir.AluOpType.mult)
                    f_t = f_pool.tile([P, c], _FP8, tag="ff")
                    nc.scalar.mul(f_t[:], e_t[:], s_t[:])
                    F.append(f_t)
                state[b]["F"] = F

            def emit_ft(b):
                F = state[b]["F"]
                FT = ft_pool.tile([P, cm, c], _FP8, tag="ft", name=f"FT_{b}")
                for i in range(cm):
                    pe = pE_pool.tile([P, cm, P, 2], _FP8, tag="pE")
                    for mc in range(cm):
                        nc.tensor.transpose(
                            pe[:, mc, :, 0],
                            F[mc][:, i * P:(i + 1) * P],
                            ident8[:],
                        )
                    if i % 2 == 0:
                        nc.vector.tensor_copy(FT[:, i, :], pe[:, :, :, 0])
                    else:
                        nc.scalar.copy(FT[:, i, :], pe[:, :, :, 0])
                state[b]["FT"] = FT

            def emit_mm2_epilogue(b):
                FT = state[b]["FT"]
                x8 = state[b]["x8"]
                for m in range(cm):
                    for q in range(ln // 2):
                        stg = st_pool.tile([P, 2 * NT], _F32, tag="stg")
                        for sub in range(2):
                            n = 2 * q + sub
                            pr = pR_pool.tile([P, NT], _F32, tag="pR")
                            for t in range(cm // 2):
                                nc.tensor.matmul(
                                    pr[:],
                                    FT[:, 2 * t:2 * t + 2,
                                       m * P:(m + 1) * P],
                                    x8[:, 2 * t:2 * t + 2,
                                       n * NT:(n + 1) * NT],
                                    start=(t == 0),
                                    stop=(t == cm // 2 - 1),
                                    perf_mode=_DR,
                                )
                            dst = stg[:, sub * NT:(sub + 1) * NT]
                            xs = xf_slice(b, m, n * NT, NT)
                            ei = state[b]["ep_ci"] = state[b].get(
                                "ep_ci", -1) + 1
                            if ei % 8 in (2, 5, 7):
                                nc.scalar.copy(dst, pr[:])
                                nc.gpsimd.tensor_tensor(
                                    dst, dst, xs, op=mybir.AluOpType.add)
                            else:
                                nc.vector.tensor_tensor(
                                    dst, pr[:], xs, op=mybir.AluOpType.add)
                        # issue stores from gpsimd's DGE ring so prefetch
                        # loads queued on sync can't delay them
                        nc.gpsimd.dma_start(
                            out_d[b, m * P:(m + 1) * P,
                                  2 * q * NT:2 * (q + 1) * NT],
                            stg[:])
                del state[b]["xf"], state[b]["x8"]

            # PE stream per iteration: mm1(b), T(b+1), FT(b), mm2(b) —
            # the next batch's transposes fill the PE while softmax(b)
            # runs on DVE/ACT, so FT(b) is ready when the PE reaches it.
            emit_load_cast(0)
            emit_transpose(0)
            for b in range(nb):
                emit_mm1_softmax(b)
                if b + 1 < nb:
                    emit_load_cast(b + 1)
                    emit_transpose(b + 1)
                emit_ft(b)
                emit_mm2_epilogue(b)
    nc.compile()
    return nc


_NC_CACHE = {}


def _get_nc():
    if "nc" not in _NC_CACHE:
        _NC_CACHE["nc"] = build_nc()
    return _NC_CACHE["nc"]


def _make_consts(eta: np.ndarray):
    eta128 = np.ascontiguousarray(
        np.broadcast_to(eta.reshape(1, 1).astype(np.float32), (P, 1)))
    ident8 = np.eye(P, dtype=ml_dtypes.float8_e4m3)
    ident16 = np.eye(P, dtype=ml_dtypes.bfloat16)
    return eta128, ident8, ident16


def kernel(minibatch: np.ndarray, eta: np.ndarray) -> np.ndarray:
    from concourse.bass_utils import run_bass_kernel_spmd

    assert minibatch.shape == (B, C, L)
    nc = _get_nc()
    eta128, ident8, ident16 = _make_consts(eta)
    in_maps = []
    for i in range(N_CORES):
        in_maps.append({
            "x": np.ascontiguousarray(
                minibatch[i * NB:(i + 1) * NB].astype(np.float32)),
            "eta128": eta128,
            "ident8": ident8,
            "ident16": ident16,
        })
    res = run_bass_kernel_spmd(nc, in_maps, core_ids=list(range(N_CORES)))
    out = np.concatenate([res.results[i]["out"] for i in range(N_CORES)],
                         axis=0)
    return out.astype(np.float32)


# revision 17
# speedup vs baseline: 1.0092x; 1.0092x over previous
"""Trainium2 Bass kernel for nn_ConvchannelAttentionBlock.

reference (per batch b):
    S      = x @ x.T                      (C x C, symmetric; contraction over L)
    probs  = softmax(rowmax(S) - S)       == exp(rowmin(S) - S) / rowsum(...)
    read   = probs @ x                    (C x L)
    out    = eta * read + x

Sharding: data-parallel over B. Each of the 8 cores gets 4 batches and
runs an identical NEFF (SPMD) on its shard; outputs are concatenated.

fp8 pipeline (per batch), software-pipelined across batches by the Tile
scheduler; all matmuls run in float8e4 with MatmulPerfMode.DoubleRow
(256-deep contraction per instruction, 2x PE throughput vs bf16):
  1. DMA x (f32) in [128,1024] tiles -> SBUF; cast f32->fp8 into X8
     ([128, cm, L], row-block index on dim1) on DVE/ACT/GPSIMD.
  2. Build XT ([128, lk, C], l-block index on dim1, fp8) via PE
     transposes (128x128 blocks, 8 per PSUM stage) -> SBUF.
  3. mm1: upper-triangular blocks of S = x @ x.T via DoubleRow matmuls
     over 16 k-pairs into PSUM (f32); rounded to bf16 in SBUF; lower
     blocks mirrored via bf16 PE transposes.
  4. softmax: rowmin on DVE, E = exp(rowmin - S) on ACT (bf16) with
     fused row-sum Z; s = eta/Z; F = s*E cast to fp8 on ACT. (eta == 0
     gives F == 0 exactly, so out == x exactly.)
  5. F^T via fp8 PE transposes -> FT ([128, cm, C], d-block on dim1).
  6. mm2: R = F^T.T @ X8 via DoubleRow matmuls (2 k-pairs) into PSUM.
  7. out = R + x: half the tiles DVE-add (PSUM+SBUF), half ACT-copy +
     GPSIMD-add; staged [128,1024] f32, DMA out.
"""

import sys

if "/opt/trn_rl_repo" not in sys.path:
    sys.path.insert(0, "/opt/trn_rl_repo")

import numpy as np
import ml_dtypes

import concourse.bacc as bacc
import concourse.tile as tile
from concourse import mybir

B, C, L = 32, 512, 4096
N_CORES = 8
NB = B // N_CORES  # batches per core
P = 128            # partitions
NT = 512           # matmul moving free dim / PSUM bank (f32)
CH = 1024          # xf load/cast chunk

_F32 = mybir.dt.float32
_BF16 = mybir.dt.bfloat16
_FP8 = mybir.dt.float8e4
_DR = mybir.MatmulPerfMode.DoubleRow


def build_nc(nb=NB, c=C, l=L):
    """Build the per-core fp8 Bass kernel (nb batches of [c, l])."""
    cm = c // P            # 4 row blocks
    lk = l // P            # 32 l blocks
    ln = l // NT           # 8 mm2 column tiles
    nch = l // CH          # 4 chunks per row block
    TG = 8                 # transposes grouped per PSUM stage

    nc = bacc.Bacc("TRN2", target_bir_lowering=False, debug=False)
    x_d = nc.dram_tensor("x", [nb, c, l], _F32, kind="ExternalInput").ap()
    eta_d = nc.dram_tensor("eta128", [P, 1], _F32, kind="ExternalInput").ap()
    id8_d = nc.dram_tensor("ident8", [P, P], _FP8, kind="ExternalInput").ap()
    id16_d = nc.dram_tensor("ident16", [P, P], _BF16,
                            kind="ExternalInput").ap()
    out_d = nc.dram_tensor("out", [nb, c, l], _F32, kind="ExternalOutput").ap()

    with tile.TileContext(nc) as tc:
        with (
            tc.tile_pool(name="const", bufs=1) as const_pool,
            tc.tile_pool(name="xf", bufs=24) as xf_pool,
            tc.tile_pool(name="x8", bufs=2) as x8_pool,
            tc.tile_pool(name="xT", bufs=2) as xT_pool,
            tc.tile_pool(name="ssb", bufs=6) as ssb_pool,
            tc.tile_pool(name="ee", bufs=3) as e_pool,
            tc.tile_pool(name="ff", bufs=6) as f_pool,
            tc.tile_pool(name="ft", bufs=2) as ft_pool,
            tc.tile_pool(name="stg", bufs=4) as st_pool,
            tc.tile_pool(name="stat", bufs=20) as stat_pool,
            tc.tile_pool(name="pT", bufs=2, space="PSUM") as pT_pool,
            tc.tile_pool(name="pS", bufs=2, space="PSUM") as pS_pool,
            tc.tile_pool(name="pE", bufs=1, space="PSUM") as pE_pool,
            tc.tile_pool(name="pR", bufs=3, space="PSUM") as pR_pool,
        ):
            ident8 = const_pool.tile([P, P], _FP8, tag="id8")
            nc.sync.dma_start(ident8[:], id8_d[:, :])
            ident16 = const_pool.tile([P, P], _BF16, tag="id16")
            nc.sync.dma_start(ident16[:], id16_d[:, :])
            eta = const_pool.tile([P, 1], _F32, tag="eta")
            nc.sync.dma_start(eta[:], eta_d[:, :])

            state = {}

            def emit_load_cast_transpose(b):
                # per [128,1024] chunk: DMA load -> cast f32->fp8 -> 8 PE
                # transposes -> PSUM->SBUF copy, with cast and copy on
                # opposite engines so the XT copies are never queued
                # behind a run of casts (they gate the PE stream).
                xf = [[None] * nch for _ in range(cm)]
                x8 = x8_pool.tile([P, cm, l], _FP8, tag="x8", name=f"x8_{b}")
                XT = xT_pool.tile([P, lk, c], _FP8, tag="xT", name=f"XT_{b}")
                ci = 0
                pend = None  # defer each copy one unit so engine queues
                # stay [cast, copy] and never gate the PE transposes
                for m in range(cm):
                    for h in range(nch):
                        t = xf_pool.tile([P, CH], _F32, tag="xf",
                                         name=f"xf_{b}_{m}_{h}")
                        nc.sync.dma_start(
                            t[:], x_d[b, m * P:(m + 1) * P,
                                      h * CH:(h + 1) * CH])
                        xf[m][h] = t
                        dst = x8[:, m, h * CH:(h + 1) * CH]
                        # gpsimd fp8 casts measured 2.5x slower than DVE/ACT
                        if ci % 2 == 0:
                            nc.vector.tensor_copy(dst, t[:])
                        else:
                            nc.scalar.copy(dst, t[:])
                        if pend is not None:
                            src, tdst, par = pend
                            if par == 0:
                                nc.vector.tensor_copy(tdst, src)
                            else:
                                nc.scalar.copy(tdst, src)
                        # fp8 PE transpose writes PSUM at element step 2
                        pt = pT_pool.tile([P, TG, P, 2], _FP8, tag="pT")
                        for i in range(TG):
                            kb = TG * h + i
                            nc.tensor.transpose(
                                pt[:, i, :, 0],
                                x8[:, m, kb * P:(kb + 1) * P],
                                ident8[:],
                            )
                        pend = (pt[:, :, :, 0],
                                XT[:, TG * h:TG * (h + 1),
                                   m * P:(m + 1) * P],
                                ci % 2)
                        ci += 1
                src, tdst, par = pend
                if par == 0:
                    nc.vector.tensor_copy(tdst, src)
                else:
                    nc.scalar.copy(tdst, src)
                state[b] = {"xf": xf, "x8": x8, "XT": XT}

            def xf_slice(b, m, lo, width):
                h = lo // CH
                assert (lo + width - 1) // CH == h
                return state[b]["xf"][m][h][:, lo - h * CH:lo - h * CH + width]

            def emit_mm1_softmax(b):
                XT = state[b]["XT"]
                # S is symmetric: compute only upper-triangular blocks
                # (cols >= m*P for row-block m), round to bf16 in SBUF, and
                # fill lower blocks by PE-transposing the mirrored ones.
                S_sb = [ssb_pool.tile([P, c], _BF16, tag="ssb",
                                      name=f"Ssb_{b}_{m}")
                        for m in range(cm)]
                F = []
                for m in range(cm):
                    lo = m * P
                    ps = pS_pool.tile([P, NT], _F32, tag="pS")
                    for t in range(lk // 2):
                        nc.tensor.matmul(
                            ps[:, lo:c],
                            XT[:, 2 * t:2 * t + 2, m * P:(m + 1) * P],
                            XT[:, 2 * t:2 * t + 2, lo:c],
                            start=(t == 0),
                            stop=(t == lk // 2 - 1),
                            perf_mode=_DR,
                        )
                    if m % 2 == 0:
                        nc.vector.tensor_copy(S_sb[m][:, lo:c], ps[:, lo:c])
                    else:
                        nc.scalar.copy(S_sb[m][:, lo:c], ps[:, lo:c])
                    # mirror block (m, m2) for every later row-block m2
                    for m2 in range(m + 1, cm):
                        ptx = pE_pool.tile([P, P], _BF16, tag="pE")
                        nc.tensor.transpose(
                            ptx[:],
                            S_sb[m][:, m2 * P:(m2 + 1) * P],
                            ident16[:],
                        )
                        if m2 % 2 == 0:
                            nc.vector.tensor_copy(
                                S_sb[m2][:, lo:lo + P], ptx[:])
                        else:
                            nc.scalar.copy(S_sb[m2][:, lo:lo + P], ptx[:])
                for m in range(cm):
                    mn = stat_pool.tile([P, 1], _F32, tag="stat")
                    nc.vector.tensor_reduce(
                        mn[:], S_sb[m][:], axis=mybir.AxisListType.X,
                        op=mybir.AluOpType.min)
                    e_t = e_pool.tile([P, c], _BF16, tag="ee")
                    z_t = stat_pool.tile([P, 1], _F32, tag="stat")
                    nc.scalar.activation(
                        e_t[:], S_sb[m][:], mybir.ActivationFunctionType.Exp,
                        bias=mn[:], scale=-1.0, accum_out=z_t[:])
                    r_t = stat_pool.tile([P, 1], _F32, tag="stat")
                    nc.vector.reciprocal(r_t[:], z_t[:])
                    s_t = stat_pool.tile([P, 1], _F32, tag="stat")
                    nc.vector.tensor_tensor(
                        s_t[:], eta[:], r_t[:], op=mybir.AluOpType.mult)
                    f_t = f_pool.tile([P, c], _FP8, tag="ff")
                    nc.scalar.mul(f_t[:], e_t[:], s_t[:])
                    F.append(f_t)
                state[b]["F"] = F

            def emit_ft(b):
                F = state[b]["F"]
                FT = ft_pool.tile([P, cm, c], _FP8, tag="ft", name=f"FT_{b}")
                for i in range(cm):
                    pe = pE_pool.tile([P, cm, P, 2], _FP8, tag="pE")
                    for mc in range(cm):
                        nc.tensor.transpose(
                            pe[:, mc, :, 0],
                            F[mc][:, i * P:(i + 1) * P],
                            ident8[:],
                        )
                    if i % 2 == 0:
                        nc.vector.tensor_copy(FT[:, i, :], pe[:, :, :, 0])
                    else:
                        nc.scalar.copy(FT[:, i, :], pe[:, :, :, 0])
                state[b]["FT"] = FT

            def emit_mm2_epilogue(b):
                FT = state[b]["FT"]
                x8 = state[b]["x8"]
                for m in range(cm):
                    for q in range(ln // 2):
                        stg = st_pool.tile([P, 2 * NT], _F32, tag="stg")
                        for sub in range(2):
                            n = 2 * q + sub
                            pr = pR_pool.tile([P, NT], _F32, tag="pR")
                            for t in range(cm // 2):
                                nc.tensor.matmul(
                                    pr[:],
                                    FT[:, 2 * t:2 * t + 2,
                                       m * P:(m + 1) * P],
                                    x8[:, 2 * t:2 * t + 2,
                                       n * NT:(n + 1) * NT],
                                    start=(t == 0),
                                    stop=(t == cm // 2 - 1),
                                    perf_mode=_DR,
                                )
                            dst = stg[:, sub * NT:(sub + 1) * NT]
                            xs = xf_slice(b, m, n * NT, NT)
                            ei = (4 * q + 2 * sub + m) % 4
                            if ei != 3:
                                nc.vector.tensor_tensor(
                                    dst, pr[:], xs, op=mybir.AluOpType.add)
                            else:
                                nc.scalar.copy(dst, pr[:])
                                nc.gpsimd.tensor_tensor(
                                    dst, dst, xs, op=mybir.AluOpType.add)
                        # issue stores from gpsimd's DGE ring so prefetch
                        # loads queued on sync can't delay them
                        nc.gpsimd.dma_start(
                            out_d[b, m * P:(m + 1) * P,
                                  2 * q * NT:2 * (q + 1) * NT],
                            stg[:])
                del state[b]["xf"], state[b]["x8"]

            # PE stream per iteration: mm1(b), T(b+1), FT(b), mm2(b) —
            # the next batch's transposes fill the PE while softmax(b)
            # runs on DVE/ACT, so FT(b) is ready when the PE reaches it.
            emit_load_cast_transpose(0)
            for b in range(nb):
                emit_mm1_softmax(b)
                if b + 1 < nb:
                    emit_load_cast_transpose(b + 1)
                emit_ft(b)
                emit_mm2_epilogue(b)
    nc.compile()
    return nc


_NC_CACHE = {}


def _get_nc():
    if "nc" not in _NC_CACHE:
        _NC_CACHE["nc"] = build_nc()
    return _NC_CACHE["nc"]


def _make_consts(eta: np.ndarray):
    eta128 = np.ascontiguousarray(
        np.broadcast_to(eta.reshape(1, 1).astype(np.float32), (P, 1)))
    ident8 = np.eye(P, dtype=ml_dtypes.float8_e4m3)
    ident16 = np.eye(P, dtype=ml_dtypes.bfloat16)
    return eta128, ident8, ident16


def kernel(minibatch: np.ndarray, eta: np.ndarray) -> np.ndarray:
    from concourse.bass_utils import run_bass_kernel_spmd

    assert minibatch.shape == (B, C, L)
    nc = _get_nc()
    eta128, ident8, ident16 = _make_consts(eta)
    in_maps = []
    for i in range(N_CORES):
        in_maps.append({
            "x": np.ascontiguousarray(
                minibatch[i * NB:(i + 1) * NB].astype(np.float32)),
            "eta128": eta128,
            "ident8": ident8,
            "ident16": ident16,
        })
    res = run_bass_kernel_spmd(nc, in_maps, core_ids=list(range(N_CORES)))
    out = np.concatenate([res.results[i]["out"] for i in range(N_CORES)],
                         axis=0)
    return out.astype(np.float32)
